# revision 3
# baseline (speedup 1.0000x reference)
"""MinLSTM layer on 8 Trainium2 NeuronCores.

Math (equivalent to the log-space reference, done in linear space):
    f_pre = x @ W_f.T + b_f ; i_pre = x @ W_i.T + b_i ; h_pre = x @ W_h.T + b_h
    sf = sigmoid(f_pre) ; si = sigmoid(i_pre)
    f = sf / (sf + si)                       # normalized forget gate
    i = 1 - f                                # = si / (sf + si)
    g = max(sigmoid(h_pre), h_pre + 0.5)     # == exp(log_g), exactly
    h_t = f_t * h_{t-1} + i_t * g_t,  h_0 = 1
The gates satisfy f in (0,1), g > 0, so h stays in a tame range and the
recurrence is numerically stable in fp32 (validated: max rel err vs the fp32
log-space reference ~5e-4 for exact matmuls, which is the reference's own
fp32 noise floor).

Sharding: 8 cores = batch(4) x hidden-halves(2). Core c handles batch b=c//2,
hidden slice [(c%2)*512, (c%2+1)*512). No cross-core communication; the scan
runs along T inside each core via the DVE TensorTensorScan instruction
(state = f*state - mv per step, mv = (f-1)*g = -i*g).

Device layout: gates computed as [h_part, t_free] via out = W_sliceT.T @ xT;
host pre-transposes x and W (numpy) and re-transposes the [512, 4096] per-core
output back to [T, Dh]. Matmuls run in 512-wide t-chunks (one PSUM bank);
elementwise+scan run in 1024-wide t-chunks to amortize DVE fixed overhead.
The scan is chunked along T with the carry passed as the previous chunk's
last column.
"""

import sys

for _p in ("/opt/trn_rl_repo",):
    if _p not in sys.path:
        sys.path.append(_p)

import numpy as np

import concourse.bass as bass
import concourse.tile as tile
from concourse import bacc, mybir
from concourse.bass_utils import run_bass_kernel_spmd

B, T, DIN, DH = 4, 4096, 1024, 1024
N_CORES = 8
HSH = DH // 2          # 512 hidden channels per core
P = 128                # partitions
KT = DIN // P          # 8 contraction tiles
NT = 512               # matmul t-chunk (free dim, one PSUM bank)
NE = 1024              # elementwise/scan t-chunk
JT = T // NE           # 4 super-chunks
IT = HSH // P          # 4 h-tiles per core

# float32r streams fp32 operands through the PE at bf16 rate when the moving
# free dim >= 256. Measured (K=128): mean rel err ~1e-3 vs fp64, ~16x better
# than bf16. Fallbacks: mybir.dt.float32 (4x slower, exact) / bfloat16.
MM_DT = mybir.dt.float32r

_COMPILED = None


def _build():
    AF = mybir.ActivationFunctionType
    OP = mybir.AluOpType
    f32 = mybir.dt.float32

    nc = bacc.Bacc("TRN2", target_bir_lowering=False, debug=False)

    xT = nc.dram_tensor("xT", [DIN, T], MM_DT, kind="ExternalInput").ap()
    wf = nc.dram_tensor("wf", [DIN, HSH], MM_DT, kind="ExternalInput").ap()
    wi = nc.dram_tensor("wi", [DIN, HSH], MM_DT, kind="ExternalInput").ap()
    wh = nc.dram_tensor("wh", [DIN, HSH], MM_DT, kind="ExternalInput").ap()
    # packed per-partition scalars: [b_f | b_i | b_h | b_h+0.5], each (128, IT)
    biases = nc.dram_tensor("biases", [P, 4 * IT], f32, kind="ExternalInput").ap()
    out = nc.dram_tensor("out", [HSH, T], f32, kind="ExternalOutput").ap()

    # DRAM views: (KT*P, n) -> [p, k, n] so one DMA fills a [128, KT, n] tile
    xT_v = xT.rearrange("(k p) t -> p k t", p=P)
    w_v = {g: w.rearrange("(k p) h -> p k h", p=P)
           for g, w in (("f", wf), ("i", wi), ("h", wh))}

    with tile.TileContext(nc) as tc:
        with (
            tc.tile_pool(name="wpool", bufs=1) as wpool,
            tc.tile_pool(name="bpool", bufs=1) as bpool,
            tc.tile_pool(name="xpool", bufs=3) as xpool,
            tc.tile_pool(name="psum", bufs=2, space="PSUM") as pspool,
            tc.tile_pool(name="work", bufs=2) as work,
            tc.tile_pool(name="hpool", bufs=2) as hpool,
        ):
            bias_t = bpool.tile([P, 4 * IT], f32, tag="bias")
            nc.sync.dma_start(out=bias_t[:], in_=biases[:])
            wt = {}
            for g in ("f", "i", "h"):
                wt[g] = wpool.tile([P, KT, HSH], MM_DT, tag=f"w{g}", name=f"w{g}_t")
                nc.sync.dma_start(out=wt[g][:], in_=w_v[g][:])

            def bias_ap(kind, i):
                return bias_t[:, kind * IT + i:kind * IT + i + 1]

            hprev = [None] * IT
            for J in range(JT):
                xc = []
                for half in range(2):
                    t0 = J * NE + half * NT
                    xch = xpool.tile([P, KT, NT], MM_DT, tag="xc", name="xc_t")
                    nc.sync.dma_start(out=xch[:], in_=xT_v[:, :, t0:t0 + NT])
                    xc.append(xch)
                for i in range(IT):
                    hsl = slice(i * P, (i + 1) * P)
                    sf = work.tile([P, NE], f32, tag="sf")
                    si = work.tile([P, NE], f32, tag="si")
                    sg = work.tile([P, NE], f32, tag="sg")
                    gt = work.tile([P, NE], f32, tag="gt")
                    for half in range(2):
                        esl = slice(half * NT, (half + 1) * NT)
                        ps = {}
                        for g in ("f", "i", "h"):
                            ps[g] = pspool.tile([P, NT], f32, tag=f"ps{g}",
                                                name=f"ps{g}_t")
                            for k in range(KT):
                                nc.tensor.matmul(
                                    ps[g][:],
                                    lhsT=wt[g][:, k, hsl],
                                    rhs=xc[half][:, k, :],
                                    start=(k == 0),
                                    stop=(k == KT - 1),
                                )
                        nc.scalar.activation(sf[:, esl], ps["f"][:], AF.Sigmoid,
                                             bias=bias_ap(0, i), scale=1.0)
                        nc.scalar.activation(si[:, esl], ps["i"][:], AF.Sigmoid,
                                             bias=bias_ap(1, i), scale=1.0)
                        nc.scalar.activation(sg[:, esl], ps["h"][:], AF.Sigmoid,
                                             bias=bias_ap(2, i), scale=1.0)
                        # g = (h_pre_mm + (b_h + 0.5)) max sigmoid(h_pre + b_h)
                        nc.vector.scalar_tensor_tensor(
                            gt[:, esl], ps["h"][:], bias_ap(3, i), sg[:, esl],
                            op0=OP.add, op1=OP.max)
                    s = work.tile([P, NE], f32, tag="s")
                    nc.vector.tensor_add(s[:], sf[:], si[:])
                    r = work.tile([P, NE], f32, tag="r")
                    nc.vector.reciprocal_approx_fast(out=r[:], in_=s[:])
                    fc = work.tile([P, NE], f32, tag="fc")
                    nc.vector.tensor_mul(fc[:], sf[:], r[:])
                    # mv = (f - 1) * g = -i*g
                    mv = work.tile([P, NE], f32, tag="mv")
                    nc.vector.scalar_tensor_tensor(
                        mv[:], fc[:], 1.0, gt[:], op0=OP.subtract, op1=OP.mult)
                    # state = f*state - mv  (== f*state + i*g)
                    hc = hpool.tile([P, NE], f32, tag=f"h{i}", name=f"h{i}_t")
                    init = 1.0 if J == 0 else hprev[i][:, NE - 1:NE]
                    nc.vector.tensor_tensor_scan(
                        hc[:], fc[:], mv[:], init, op0=OP.mult, op1=OP.subtract)
                    hprev[i] = hc
                    nc.sync.dma_start(out=out[hsl, J * NE:(J + 1) * NE], in_=hc[:])

    nc.compile()
    return nc


def _in_maps(x, W_f, b_f, W_i, b_i, W_h, b_h):
    x = np.asarray(x, np.float32)
    wT = {g: np.ascontiguousarray(np.asarray(w, np.float32).T)
          for g, w in (("f", W_f), ("i", W_i), ("h", W_h))}
    bs = {g: np.asarray(b, np.float32) for g, b in (("f", b_f), ("i", b_i), ("h", b_h))}

    maps = []
    for c in range(N_CORES):
        b, hh = divmod(c, 2)
        hsl = slice(hh * HSH, (hh + 1) * HSH)
        bias_pack = np.concatenate([
            bs["f"][hsl].reshape(IT, P).T,
            bs["i"][hsl].reshape(IT, P).T,
            bs["h"][hsl].reshape(IT, P).T,
            (bs["h"][hsl] + 0.5).reshape(IT, P).T,
        ], axis=1)
        maps.append({
            "xT": np.ascontiguousarray(x[b].T),
            "wf": np.ascontiguousarray(wT["f"][:, hsl]),
            "wi": np.ascontiguousarray(wT["i"][:, hsl]),
            "wh": np.ascontiguousarray(wT["h"][:, hsl]),
            "biases": np.ascontiguousarray(bias_pack, dtype=np.float32),
        })
    return maps


def kernel(x, W_f, b_f, W_i, b_i, W_h, b_h):
    global _COMPILED
    if _COMPILED is None:
        _COMPILED = _build()
    nc = _COMPILED

    res = run_bass_kernel_spmd(
        nc, _in_maps(x, W_f, b_f, W_i, b_i, W_h, b_h), list(range(N_CORES)))

    full = np.empty((B, T, DH), np.float32)
    for c in range(N_CORES):
        b, hh = divmod(c, 2)
        full[b, :, hh * HSH:(hh + 1) * HSH] = res.results[c]["out"].T
    return full


# revision 4
# speedup vs baseline: 1.0621x; 1.0621x over previous
"""MinLSTM layer on 8 Trainium2 NeuronCores.

Math (equivalent to the log-space reference, done in linear space):
    f_pre = x @ W_f.T + b_f ; i_pre = x @ W_i.T + b_i ; h_pre = x @ W_h.T + b_h
    sf = sigmoid(f_pre) ; si = sigmoid(i_pre)
    f = sf / (sf + si)                       # normalized forget gate
    i = 1 - f                                # = si / (sf + si)
    g = max(sigmoid(h_pre), h_pre + 0.5)     # == exp(log_g), exactly
    h_t = f_t * h_{t-1} + i_t * g_t,  h_0 = 1
The gates satisfy f in (0,1), g > 0, so h stays in a tame range and the
recurrence is numerically stable in fp32 (max rel err vs the fp32 log-space
reference ~6e-4 = the reference's own fp32 noise floor).

Sharding: 8 cores = batch(4) x hidden-halves(2). Core c handles batch b=c//2,
hidden slice [(c%2)*512, (c%2+1)*512). No cross-core communication; the scan
runs along T inside each core via the DVE TensorTensorScan instruction
(state = f*state - mv per step, mv = (f-1)*g = -i*g).

Device layout: gates computed as [h_part, t_free] via out = W_sliceT.T @ xT;
host pre-transposes x and W (numpy) and re-transposes the [512, 4096] per-core
output back to [T, Dh]. Matmuls run in 512-wide t-chunks (one PSUM bank);
elementwise+scan run in up-to-1024-wide t-chunks to amortize DVE overhead,
with the scan carry passed as the previous chunk's last column.

Startup: the PE can only consume data as fast as HBM delivers it (~390 GB/s),
so the first super-chunk is emitted gate-major with k-sliced DMAs in priority
order [x(k), W_f(k)] -> W_i(k) -> x2(k) -> W_h(k); the PE chases the DMA
stream from ~1.3us instead of idling ~34us for all weights.
"""

import sys

for _p in ("/opt/trn_rl_repo",):
    if _p not in sys.path:
        sys.path.append(_p)

import numpy as np

import concourse.bass as bass
import concourse.tile as tile
from concourse import bacc, mybir
from concourse.bass_utils import run_bass_kernel_spmd

B, T, DIN, DH = 4, 4096, 1024, 1024
N_CORES = 8
HSH = DH // 2          # 512 hidden channels per core
P = 128                # partitions
KT = DIN // P          # 8 contraction tiles
NT = 512               # matmul t-chunk (free dim, one PSUM bank)
IT = HSH // P          # 4 h-tiles per core
# elementwise/scan super-chunks (start, length); tail chunks smaller to
# shrink the end-of-kernel DVE drain
CHUNKS = [(0, 1024), (1024, 1024), (2048, 1024), (3072, 512), (3584, 512)]

# float32r streams fp32 operands through the PE at bf16 rate when the moving
# free dim >= 256. Measured (K=128): mean rel err ~1e-3 vs fp64, ~16x better
# than bf16. Fallbacks: mybir.dt.float32 (4x slower, exact) / bfloat16.
MM_DT = mybir.dt.float32r

_COMPILED = None


def _build():
    AF = mybir.ActivationFunctionType
    OP = mybir.AluOpType
    f32 = mybir.dt.float32

    nc = bacc.Bacc("TRN2", target_bir_lowering=False, debug=False)

    xT = nc.dram_tensor("xT", [DIN, T], MM_DT, kind="ExternalInput").ap()
    wf = nc.dram_tensor("wf", [DIN, HSH], MM_DT, kind="ExternalInput").ap()
    wi = nc.dram_tensor("wi", [DIN, HSH], MM_DT, kind="ExternalInput").ap()
    wh = nc.dram_tensor("wh", [DIN, HSH], MM_DT, kind="ExternalInput").ap()
    # packed per-partition scalars: [b_f | b_i | b_h | b_h+0.5], each (128, IT)
    biases = nc.dram_tensor("biases", [P, 4 * IT], f32, kind="ExternalInput").ap()
    out = nc.dram_tensor("out", [HSH, T], f32, kind="ExternalOutput").ap()

    # DRAM views: (KT*P, n) -> [p, k, n] so one DMA fills a [128, KT, n] tile
    xT_v = xT.rearrange("(k p) t -> p k t", p=P)
    w_v = {g: w.rearrange("(k p) h -> p k h", p=P)
           for g, w in (("f", wf), ("i", wi), ("h", wh))}

    with tile.TileContext(nc) as tc:
        with (
            tc.tile_pool(name="wpool", bufs=1) as wpool,
            tc.tile_pool(name="bpool", bufs=1) as bpool,
            tc.tile_pool(name="xpool", bufs=3) as xpool,
            tc.tile_pool(name="psum", bufs=7, space="PSUM") as pspool,
            tc.tile_pool(name="work", bufs=4) as work,
            tc.tile_pool(name="hpool", bufs=2) as hpool,
        ):
            bias_t = bpool.tile([P, 4 * IT], f32, tag="bias")
            nc.sync.dma_start(out=bias_t[:], in_=biases[:])

            wt = {g: wpool.tile([P, KT, HSH], MM_DT, tag=f"w{g}", name=f"w{g}_t")
                  for g in ("f", "i", "h")}

            def bias_ap(kind, i):
                return bias_t[:, kind * IT + i:kind * IT + i + 1]

            def dma_k_sliced(dst, src_view, t0=None, nt=None):
                for k in range(KT):
                    if t0 is None:
                        nc.sync.dma_start(out=dst[:, k, :], in_=src_view[:, k, :])
                    else:
                        nc.sync.dma_start(out=dst[:, k, :],
                                          in_=src_view[:, k, t0:t0 + nt])

            def x_tile(t0):
                xc = xpool.tile([P, KT, NT], MM_DT, tag="xc", name="xc_t")
                dma_k_sliced(xc, xT_v, t0, NT)
                return xc

            def mm_block(psts, gate, xc, hsl_list, k_outer):
                """8 accumulating matmuls into each psum tile in psts."""
                if k_outer:
                    for k in range(KT):
                        for pst, hsl in zip(psts, hsl_list):
                            nc.tensor.matmul(
                                pst[:], lhsT=wt[gate][:, k, hsl], rhs=xc[:, k, :],
                                start=(k == 0), stop=(k == KT - 1))
                else:
                    for pst, hsl in zip(psts, hsl_list):
                        for k in range(KT):
                            nc.tensor.matmul(
                                pst[:], lhsT=wt[gate][:, k, hsl], rhs=xc[:, k, :],
                                start=(k == 0), stop=(k == KT - 1))

            def chain(i, sf, si, sg, gt, J, t0, ne):
                """Normalize gates, build -i*g, scan, and store chunk."""
                hsl = slice(i * P, (i + 1) * P)
                nc.vector.tensor_add(si[:], sf[:], si[:])          # s = sf+si
                r = work.tile([P, ne], f32, tag="sg", name="r_t")  # sg slot free
                nc.vector.reciprocal_approx_fast(out=r[:], in_=si[:])
                nc.vector.tensor_mul(sf[:], sf[:], r[:])           # f
                nc.vector.scalar_tensor_tensor(                    # mv = (f-1)*g
                    gt[:], sf[:], 1.0, gt[:], op0=OP.subtract, op1=OP.mult)
                hc = hpool.tile([P, ne], f32, tag=f"h{i}", name=f"h{i}_t")
                init = 1.0 if J == 0 else hprev[i][:, -1:]
                nc.vector.tensor_tensor_scan(
                    hc[:], sf[:], gt[:], init, op0=OP.mult, op1=OP.subtract)
                hprev[i] = hc
                nc.sync.dma_start(out=out[hsl, t0:t0 + ne], in_=hc[:])

            hprev = [None] * IT
            hsls = [slice(i * P, (i + 1) * P) for i in range(IT)]

            # ---- J0: gate-major, k-outer; PE chases the input DMA stream ----
            t0, ne = CHUNKS[0]
            nhalf = ne // NT
            # DMA priority order: x(h0)+W_f interleaved, W_i, x(h1), W_h
            xc0 = xpool.tile([P, KT, NT], MM_DT, tag="xc", name="xc_t")
            for k in range(KT):
                nc.sync.dma_start(out=xc0[:, k, :], in_=xT_v[:, k, t0:t0 + NT])
                nc.sync.dma_start(out=wt["f"][:, k, :], in_=w_v["f"][:, k, :])
            dma_k_sliced(wt["i"], w_v["i"])
            xcs = [xc0] + [x_tile(t0 + h * NT) for h in range(1, nhalf)]
            dma_k_sliced(wt["h"], w_v["h"])

            sf = [work.tile([P, ne], f32, tag="sf", name="sf_t") for _ in range(IT)]
            si = [work.tile([P, ne], f32, tag="si", name="si_t") for _ in range(IT)]
            sg = [work.tile([P, ne], f32, tag="sg", name="sg_t") for _ in range(IT)]
            gt = [work.tile([P, ne], f32, tag="gt", name="gt_t") for _ in range(IT)]
            for gate, dsts, bk in (("f", sf, 0), ("i", si, 1), ("h", sg, 2)):
                for half in range(nhalf):
                    esl = slice(half * NT, (half + 1) * NT)
                    psts = [pspool.tile([P, NT], f32, tag="ps", name="ps_t")
                            for _ in range(IT)]
                    mm_block(psts, gate, xcs[half], hsls, k_outer=True)
                    for i in range(IT):
                        nc.scalar.activation(dsts[i][:, esl], psts[i][:], AF.Sigmoid,
                                             bias=bias_ap(bk, i), scale=1.0)
                        if gate == "h":
                            nc.vector.scalar_tensor_tensor(
                                gt[i][:, esl], psts[i][:], bias_ap(3, i),
                                sg[i][:, esl], op0=OP.add, op1=OP.max)
            for i in range(IT):
                chain(i, sf[i], si[i], sg[i], gt[i], 0, t0, ne)

            # ---- J1+: per-htile units, weights resident ----
            for J, (t0, ne) in enumerate(CHUNKS[1:], start=1):
                xcs = [x_tile(t0 + h * NT) for h in range(ne // NT)]
                for i in range(IT):
                    sf = work.tile([P, ne], f32, tag="sf", name="sf_t")
                    si = work.tile([P, ne], f32, tag="si", name="si_t")
                    sg = work.tile([P, ne], f32, tag="sg", name="sg_t")
                    gt = work.tile([P, ne], f32, tag="gt", name="gt_t")
                    for half in range(ne // NT):
                        esl = slice(half * NT, (half + 1) * NT)
                        ps = {g: pspool.tile([P, NT], f32, tag="ps", name="ps_t")
                              for g in ("f", "i", "h")}
                        for g in ("f", "i", "h"):
                            mm_block([ps[g]], g, xcs[half], [hsls[i]], k_outer=False)
                        nc.scalar.activation(sf[:, esl], ps["f"][:], AF.Sigmoid,
                                             bias=bias_ap(0, i), scale=1.0)
                        nc.scalar.activation(si[:, esl], ps["i"][:], AF.Sigmoid,
                                             bias=bias_ap(1, i), scale=1.0)
                        nc.scalar.activation(sg[:, esl], ps["h"][:], AF.Sigmoid,
                                             bias=bias_ap(2, i), scale=1.0)
                        nc.vector.scalar_tensor_tensor(
                            gt[:, esl], ps["h"][:], bias_ap(3, i), sg[:, esl],
                            op0=OP.add, op1=OP.max)
                    chain(i, sf, si, sg, gt, J, t0, ne)

    nc.compile()
    return nc


def _in_maps(x, W_f, b_f, W_i, b_i, W_h, b_h):
    x = np.asarray(x, np.float32)
    wT = {g: np.ascontiguousarray(np.asarray(w, np.float32).T)
          for g, w in (("f", W_f), ("i", W_i), ("h", W_h))}
    bs = {g: np.asarray(b, np.float32) for g, b in (("f", b_f), ("i", b_i), ("h", b_h))}

    maps = []
    for c in range(N_CORES):
        b, hh = divmod(c, 2)
        hsl = slice(hh * HSH, (hh + 1) * HSH)
        bias_pack = np.concatenate([
            bs["f"][hsl].reshape(IT, P).T,
            bs["i"][hsl].reshape(IT, P).T,
            bs["h"][hsl].reshape(IT, P).T,
            (bs["h"][hsl] + 0.5).reshape(IT, P).T,
        ], axis=1)
        maps.append({
            "xT": np.ascontiguousarray(x[b].T),
            "wf": np.ascontiguousarray(wT["f"][:, hsl]),
            "wi": np.ascontiguousarray(wT["i"][:, hsl]),
            "wh": np.ascontiguousarray(wT["h"][:, hsl]),
            "biases": np.ascontiguousarray(bias_pack, dtype=np.float32),
        })
    return maps


def kernel(x, W_f, b_f, W_i, b_i, W_h, b_h):
    global _COMPILED
    if _COMPILED is None:
        _COMPILED = _build()
    nc = _COMPILED

    res = run_bass_kernel_spmd(
        nc, _in_maps(x, W_f, b_f, W_i, b_i, W_h, b_h), list(range(N_CORES)))

    full = np.empty((B, T, DH), np.float32)
    for c in range(N_CORES):
        b, hh = divmod(c, 2)
        full[b, :, hh * HSH:(hh + 1) * HSH] = res.results[c]["out"].T
    return full
